# revision 9
# baseline (speedup 1.0000x reference)
"""F8Linear as a column-parallel hybrid fp8/bf16 GEMM across 8 NeuronCores.

y = x @ (w_f8 * w_scale).T + bias
  x: [2, 512, 4096] bf16, w_f8: [14336, 4096] f32 (fp8-representable values),
  w_scale: scalar f32, bias: [14336] f32 -> y: [2, 512, 14336] bf16

Sharding: column-parallel - each core owns 1792 out-features; x replicated;
host gathers the 8 output slices. No collectives.

Speed trick: the TRN2 PE runs fp8e4 matmuls in DoubleRow mode at 2x the bf16
k-throughput (measured 220ns per FD=512 matmul contracting 256 k, vs 215ns
per 128 k in bf16). The weights are exactly fp8-representable: w_f8/2 lands
in TRN fp8e4 range (|w|<=224<240), so the weight side is EXACT in fp8 and
one fp32 scale (2*w_scale) is applied at PSUM drain. Only x must be
quantized to e4m3 for the DoubleRow operand, which costs ~2.7% rms error if
applied to all of K - too much for the 2e-2 gate. So K is split: the first
F=7 k-tiles of 256 (1792 k) run as fp8 DoubleRow; the remaining 18 k-tiles
of 128 (2304 k) run with x in bf16 against fp8e4 stationary weights (mixed
dtype runs at full bf16 rate and is exact). Measured vs the seeded
reference: max-rel 0.0170 / rms-rel 0.0181 at F=7.

All weights ship as fp8 (7.2 MB/core vs 14.7 bf16); x ships as 1.75 MB fp8 +
4.5 MB bf16. Per (n-tile, m-chunk) PSUM group: 7 DR + 18 bf16 matmuls
(~5.4us); 28 groups ~= 151us PE stream vs 191us for pure bf16.

Structure mirrors the tuned bf16 baseline: PE warmup during entry preamble,
phase A (first 4 n-tiles, k-outer, paced by x arrival), phase B per
(n-tile) with double-buffered weight streaming. Input DMAs ride the sync
HWDGE ring; output stores ride the scalar (ACT) HWDGE ring so they never
block inbound traffic. Drain = one ScalarE activation: bf16(psum*s2 + bias).
"""

import numpy as np
import ml_dtypes

bf16 = ml_dtypes.bfloat16
e4 = ml_dtypes.float8_e4m3  # TRN FP8_EXP4 semantics (max normal 240)

NC = 8
M, K, N = 1024, 4096, 14336
NPER = N // NC  # 1792 out-features per core
NT = NPER // 128  # 14 n-tiles
F = 8  # DoubleRow k-tiles of 256 (fp8 x); k in [0, 256*F)
KF = 256 * F
TB = (K - KF) // 128  # 18 bf16 k-tiles of 128
NA = 4  # phase-A n-tiles (4 nt x 2 m-chunks = 8 PSUM banks)
MT = 2  # m-chunks of 512

_cache = {}


def _build_nc():
    import concourse.bacc as bacc
    import concourse.mybir as mybir
    import concourse.tile as tile
    from contextlib import ExitStack

    DR = mybir.MatmulPerfMode.DoubleRow
    AF = mybir.ActivationFunctionType

    nc = bacc.Bacc("TRN2", target_bir_lowering=False, debug=False)
    x8d = nc.declare_dram_parameter("x8", [F, 128, 2, M], mybir.dt.float8e4, isOutput=False)
    xbd = nc.declare_dram_parameter("xb", [128, TB, M], mybir.dt.bfloat16, isOutput=False)
    w8d = nc.declare_dram_parameter("w8", [NT, 128, F, 2, 128], mybir.dt.float8e4, isOutput=False)
    wmd = nc.declare_dram_parameter("wm", [NT, 128, TB, 128], mybir.dt.float8e4, isOutput=False)
    bgd = nc.declare_dram_parameter("bias", [128, NT], mybir.dt.float32, isOutput=False)
    scd = nc.declare_dram_parameter("sc", [128, 1], mybir.dt.float32, isOutput=False)
    yT = nc.declare_dram_parameter("yT", [NPER, M], mybir.dt.bfloat16, isOutput=True)

    with tile.TileContext(nc) as tc, ExitStack() as ctx:
        xpool = ctx.enter_context(tc.tile_pool(name="x", bufs=1))
        wapool = ctx.enter_context(tc.tile_pool(name="wa", bufs=1))
        wpool = ctx.enter_context(tc.tile_pool(name="w", bufs=3))
        bpool = ctx.enter_context(tc.tile_pool(name="b", bufs=1))
        opool = ctx.enter_context(tc.tile_pool(name="o", bufs=4))
        pspool = ctx.enter_context(tc.tile_pool(name="ps", bufs=8, space="PSUM"))

        # PE warmup: dep-free dummy matmuls run during the entry preamble +
        # first-DMA wait (~10us before real operands land) and burn the
        # cold-clock HAM window on garbage instead of real work.
        scratch = nc.alloc_sbuf_tensor("warm_src", [128, 128], mybir.dt.bfloat16)
        ps_warm = pspool.tile([128, 128], mybir.dt.float32, tag="ps")
        for _ in range(45):
            nc.tensor.matmul(ps_warm[:, :], scratch[:, :], scratch[:, :], start=True, stop=True)

        bias_sb = bpool.tile([128, NT], mybir.dt.float32)
        sc_sb = bpool.tile([128, 1], mybir.dt.float32, tag="sc")
        nc.gpsimd.dma_start(bias_sb[:], bgd[:])
        nc.gpsimd.dma_start(sc_sb[:], scd[:])

        x8_sb = xpool.tile([128, F, 2, M], mybir.dt.float8e4, tag="x8")
        xb_sb = xpool.tile([128, TB, M], mybir.dt.bfloat16, tag="xb")
        w8A = [wapool.tile([128, F, 2, 128], mybir.dt.float8e4, tag=f"w8a{j}", name=f"w8a{j}") for j in range(NA)]
        wmA = [wapool.tile([128, TB, 128], mybir.dt.float8e4, tag=f"wma{j}", name=f"wma{j}") for j in range(NA)]

        # ---- input DMA issue order, in PE consumption order. Each dma_start
        # occupies its HWDGE sequencer ~0.65us, so supply at the head is
        # ISSUE-limited, not bandwidth-limited: alternate the two HWDGE rings
        # (sync=SP, scalar=ACT) to double the issue rate, with >=256KB chunks.
        ring = [nc.sync, nc.scalar]
        ric = [0]

        def dma2(dst, src):
            ring[ric[0] & 1].dma_start(dst, src)
            ric[0] += 1

        dma2(w8A[0][:], w8d[0])
        dma2(x8_sb[:, 0], x8d[0])
        dma2(w8A[1][:], w8d[1])
        dma2(w8A[2][:], w8d[2])
        dma2(w8A[3][:], w8d[3])
        dma2(x8_sb[:, 1], x8d[1])
        dma2(x8_sb[:, 2], x8d[2])
        dma2(x8_sb[:, 3], x8d[3])
        for j in range(NA):
            dma2(wmA[j][:], wmd[j])
        for f in range(4, F):
            dma2(x8_sb[:, f], x8d[f])
        XBC = 2  # xb DMA chunk (k-tiles)
        for c0 in range(0, TB, XBC):
            c1 = min(c0 + XBC, TB)
            dma2(xb_sb[:, c0:c1, :], xbd[:, c0:c1, :])

        def mm_dr(ps, wt, f, mc, start):
            nc.tensor.matmul(
                ps[:, :],
                wt[:, f, :, :],
                x8_sb[:, f, :, mc * 512:(mc + 1) * 512],
                start=start, stop=False, perf_mode=DR,
            )

        def mm_bf(ps, wt, t, mc, stop):
            nc.tensor.matmul(
                ps[:, :],
                wt[:, t, :],
                xb_sb[:, t, mc * 512:(mc + 1) * 512],
                start=False, stop=stop,
            )

        def drain2(psums, nt):
            # both m-chunks of one n-tile -> one SBUF tile -> one store on
            # the scalar (ACT) HWDGE ring; bf16(psum * 2*w_scale + bias)
            o = opool.tile([128, M], mybir.dt.bfloat16, tag="o", name=f"o{nt}")
            for mc in range(MT):
                nc.scalar.activation(
                    o[:, mc * 512:(mc + 1) * 512], psums[mc][:, :],
                    AF.Identity, bias=bias_sb[:, nt:nt + 1], scale=sc_sb[:, 0:1],
                )
            nc.scalar.dma_start(yT[nt * 128:(nt + 1) * 128, :], o[:])

        def drain1(ps, nt, mc):
            o = opool.tile([128, 512], mybir.dt.bfloat16, tag="oh", name=f"o{nt}_{mc}")
            nc.scalar.activation(
                o[:], ps[:, :],
                AF.Identity, bias=bias_sb[:, nt:nt + 1], scale=sc_sb[:, 0:1],
            )
            nc.scalar.dma_start(yT[nt * 128:(nt + 1) * 128, mc * 512:(mc + 1) * 512], o[:])

        # ---- Phase A: nt 0..NA-1, k-outer, paced by x arrival
        psA = {
            (j, mc): pspool.tile([128, 512], mybir.dt.float32, tag="ps", name=f"psA{j}_{mc}")
            for j in range(NA) for mc in range(MT)
        }
        for f in range(F):
            for j in range(NA):
                for mc in range(MT):
                    mm_dr(psA[j, mc], w8A[j], f, mc, start=(f == 0))
        for t in range(TB):
            for j in range(NA):
                for mc in range(MT):
                    mm_bf(psA[j, mc], wmA[j], t, mc, stop=(t == TB - 1))
        for j in range(NA):
            drain2([psA[j, 0], psA[j, 1]], j)

        # ---- Phase B: per n-tile, weights double-buffered
        for nt in range(NA, NT):
            w8t = wpool.tile([128, F, 2, 128], mybir.dt.float8e4, tag="w8b", name=f"w8_{nt}")
            wmt = wpool.tile([128, TB, 128], mybir.dt.float8e4, tag="wmb", name=f"wm_{nt}")
            nc.sync.dma_start(w8t[:], w8d[nt])
            nc.sync.dma_start(wmt[:], wmd[nt])
            last = nt == NT - 1
            psb = [
                pspool.tile([128, 512], mybir.dt.float32, tag="ps", name=f"ps{nt}_{mc}")
                for mc in range(MT)
            ]
            for mc in range(MT):
                for f in range(F):
                    mm_dr(psb[mc], w8t, f, mc, start=(f == 0))
                for t in range(TB):
                    mm_bf(psb[mc], wmt, t, mc, stop=(t == TB - 1))
                if last and mc == 0:
                    drain1(psb[mc], nt, mc)
                elif last:
                    # final drain split into 256-col chunks; the two stores
                    # ride different HWDGE rings so they overlap
                    oA = opool.tile([128, 256], mybir.dt.bfloat16, tag="ohA", name="ohA")
                    oB = opool.tile([128, 256], mybir.dt.bfloat16, tag="ohB", name="ohB")
                    nc.scalar.activation(
                        oA[:], psb[mc][:, 0:256],
                        AF.Identity, bias=bias_sb[:, nt:nt + 1], scale=sc_sb[:, 0:1],
                    )
                    nc.sync.dma_start(yT[nt * 128:(nt + 1) * 128, 512:768], oA[:])
                    nc.scalar.activation(
                        oB[:], psb[mc][:, 256:512],
                        AF.Identity, bias=bias_sb[:, nt:nt + 1], scale=sc_sb[:, 0:1],
                    )
                    nc.scalar.dma_start(yT[nt * 128:(nt + 1) * 128, 768:M], oB[:])
            if not last:
                drain2(psb, nt)
    nc.compile()
    return nc


def _prep_inputs(x, weight_f8, w_scale, bias):
    x2 = np.asarray(x)
    if x2.dtype != bf16:
        x2 = x2.astype(bf16)
    xm = x2.reshape(M, K)
    xT = np.ascontiguousarray(xm.T)  # [K, M] bf16

    # x fp8 part: [f, p, j, m] with k = f*256 + j*128 + p
    x8_dev = np.ascontiguousarray(
        xT[:KF].reshape(F, 2, 128, M).transpose(0, 2, 1, 3)
    ).astype(e4)  # [F, 128, 2, M]
    # x bf16 part: [p, t, m] with k = KF + t*128 + p
    xb_dev = np.ascontiguousarray(
        xT[KF:].reshape(TB, 128, M).transpose(1, 0, 2)
    )  # [128, TB, M]

    # weights: w_f8/2 is exactly fp8e4-representable (<=224); compensated by
    # scale 2*w_scale at drain. (Sub-subnormal tail rounds with max abs err
    # 2^-10 in w_f8 units - negligible.)
    wq = np.asarray(weight_f8, dtype=np.float32)
    w_half = (wq * np.float32(0.5)).astype(e4)  # [N, K] fp8

    s2 = np.float32(2.0) * np.float32(np.asarray(w_scale).astype(bf16))
    sc_dev = np.full((128, 1), s2, np.float32)

    bias_r = np.asarray(bias, dtype=np.float32).astype(bf16).astype(np.float32)

    in_maps = []
    for c in range(NC):
        part = w_half[c * NPER:(c + 1) * NPER]  # [1792, 4096] fp8
        w8_dev = np.ascontiguousarray(
            part[:, :KF].reshape(NT, 128, F, 2, 128).transpose(0, 4, 2, 3, 1)
        )  # [NT, p, F, j, n2]
        wm_dev = np.ascontiguousarray(
            part[:, KF:].reshape(NT, 128, TB, 128).transpose(0, 3, 2, 1)
        )  # [NT, p, TB, n2]
        bias_grid = np.ascontiguousarray(
            bias_r[c * NPER:(c + 1) * NPER].reshape(NT, 128).T
        )  # [128, NT]
        in_maps.append({
            "x8": x8_dev, "xb": xb_dev, "w8": w8_dev, "wm": wm_dev,
            "bias": bias_grid, "sc": sc_dev,
        })
    return in_maps


def run(x, weight_f8, w_scale, bias, trace=False, tmpdir=None):
    from concourse.bass_utils import run_bass_kernel_spmd

    if "nc" not in _cache:
        _cache["nc"] = _build_nc()
    nc = _cache["nc"]
    in_maps = _prep_inputs(x, weight_f8, w_scale, bias)
    res = run_bass_kernel_spmd(
        nc, in_maps, list(range(NC)), trace=trace, tmpdir=tmpdir
    )
    parts = [np.asarray(res.results[c]["yT"]) for c in range(NC)]  # each [1792, 1024]
    y = np.ascontiguousarray(np.concatenate(parts, axis=0).T)  # [1024, 14336]
    return y.reshape(2, 512, N), res


def kernel(x, weight_f8, w_scale, bias):
    y, _ = run(x, weight_f8, w_scale, bias)
    return y


# revision 10
# speedup vs baseline: 1.1887x; 1.1887x over previous
"""F8Linear as a column-parallel hybrid fp8/bf16 GEMM across 8 NeuronCores.

y = x @ (w_f8 * w_scale).T + bias
  x: [2, 512, 4096] bf16, w_f8: [14336, 4096] f32 (fp8-representable values),
  w_scale: scalar f32, bias: [14336] f32 -> y: [2, 512, 14336] bf16

Sharding: column-parallel - each core owns 1792 out-features; x replicated;
host gathers the 8 output slices. No collectives.

Speed trick: the TRN2 PE runs fp8e4 matmuls in DoubleRow mode at 2x the bf16
k-throughput (measured 220ns per FD=512 matmul contracting 256 k, vs 215ns
per 128 k in bf16). The weights are exactly fp8-representable: w_f8/2 lands
in TRN fp8e4 range (|w|<=224<240), so the weight side is EXACT in fp8 and
one fp32 scale (2*w_scale) is applied at PSUM drain. Only x must be
quantized to e4m3 for the DoubleRow operand, which costs ~2.7% rms error if
applied to all of K - too much for the 2e-2 gate. So K is split: the first
F=7 k-tiles of 256 (1792 k) run as fp8 DoubleRow; the remaining 18 k-tiles
of 128 (2304 k) run with x in bf16 against fp8e4 stationary weights (mixed
dtype runs at full bf16 rate and is exact). Measured vs the seeded
reference: max-rel 0.0170 / rms-rel 0.0181 at F=7.

All weights ship as fp8 (7.2 MB/core vs 14.7 bf16); x ships as 1.75 MB fp8 +
4.5 MB bf16. Per (n-tile, m-chunk) PSUM group: 7 DR + 18 bf16 matmuls
(~5.4us); 28 groups ~= 151us PE stream vs 191us for pure bf16.

Structure mirrors the tuned bf16 baseline: PE warmup during entry preamble,
phase A (first 4 n-tiles, k-outer, paced by x arrival), phase B per
(n-tile) with double-buffered weight streaming. Input DMAs ride the sync
HWDGE ring; output stores ride the scalar (ACT) HWDGE ring so they never
block inbound traffic. Drain = one ScalarE activation: bf16(psum*s2 + bias).
"""

import numpy as np
import ml_dtypes

bf16 = ml_dtypes.bfloat16
e4 = ml_dtypes.float8_e4m3  # TRN FP8_EXP4 semantics (max normal 240)

NC = 8
M, K, N = 1024, 4096, 14336
NPER = N // NC  # 1792 out-features per core
NT = NPER // 128  # 14 n-tiles
F = 8  # DoubleRow k-tiles of 256 (fp8 x); k in [0, 256*F)
KF = 256 * F
TB = (K - KF) // 128  # 18 bf16 k-tiles of 128
NA = 4  # phase-A n-tiles (4 nt x 2 m-chunks = 8 PSUM banks)
MT = 2  # m-chunks of 512

_cache = {}


def _build_nc():
    import concourse.bacc as bacc
    import concourse.mybir as mybir
    import concourse.tile as tile
    from contextlib import ExitStack

    DR = mybir.MatmulPerfMode.DoubleRow
    AF = mybir.ActivationFunctionType

    nc = bacc.Bacc("TRN2", target_bir_lowering=False, debug=False)
    x8d = nc.declare_dram_parameter("x8", [F, 128, 2, M], mybir.dt.float8e4, isOutput=False)
    xbd = nc.declare_dram_parameter("xb", [128, TB, M], mybir.dt.bfloat16, isOutput=False)
    w8d = nc.declare_dram_parameter("w8", [NT, 128, F, 2, 128], mybir.dt.float8e4, isOutput=False)
    wmd = nc.declare_dram_parameter("wm", [NT, 128, TB, 128], mybir.dt.float8e4, isOutput=False)
    bgd = nc.declare_dram_parameter("bias", [128, NT], mybir.dt.float32, isOutput=False)
    scd = nc.declare_dram_parameter("sc", [128, 1], mybir.dt.float32, isOutput=False)
    yT = nc.declare_dram_parameter("yT", [NPER, M], mybir.dt.bfloat16, isOutput=True)

    with tile.TileContext(nc) as tc, ExitStack() as ctx:
        xpool = ctx.enter_context(tc.tile_pool(name="x", bufs=1))
        wapool = ctx.enter_context(tc.tile_pool(name="wa", bufs=1))
        wpool = ctx.enter_context(tc.tile_pool(name="w", bufs=3))
        bpool = ctx.enter_context(tc.tile_pool(name="b", bufs=1))
        opool = ctx.enter_context(tc.tile_pool(name="o", bufs=4))
        pspool = ctx.enter_context(tc.tile_pool(name="ps", bufs=8, space="PSUM"))

        # PE warmup: dep-free dummy matmuls run during the entry preamble +
        # first-DMA wait (~10us before real operands land) and burn the
        # cold-clock HAM window on garbage instead of real work.
        scratch = nc.alloc_sbuf_tensor("warm_src", [128, 128], mybir.dt.bfloat16)
        ps_warm = pspool.tile([128, 128], mybir.dt.float32, tag="ps")
        for _ in range(45):
            nc.tensor.matmul(ps_warm[:, :], scratch[:, :], scratch[:, :], start=True, stop=True)

        bias_sb = bpool.tile([128, NT], mybir.dt.float32)
        sc_sb = bpool.tile([128, 1], mybir.dt.float32, tag="sc")
        nc.gpsimd.dma_start(bias_sb[:], bgd[:])
        nc.gpsimd.dma_start(sc_sb[:], scd[:])

        x8_sb = xpool.tile([128, F, 2, M], mybir.dt.float8e4, tag="x8")
        xb_sb = xpool.tile([128, TB, M], mybir.dt.bfloat16, tag="xb")
        w8A = [wapool.tile([128, F, 2, 128], mybir.dt.float8e4, tag=f"w8a{j}", name=f"w8a{j}") for j in range(NA)]
        wmA = [wapool.tile([128, TB, 128], mybir.dt.float8e4, tag=f"wma{j}", name=f"wma{j}") for j in range(NA)]

        # ---- input DMA issue order (sync ring), matched to PE consumption.
        # Each dma_start occupies the sequencer ~0.65us, so >=256KB chunks;
        # concurrent input DMA on the scalar ring slows the PE stream ~19%
        # (SBUF port contention) - keep all inputs on the sync ring.
        nc.sync.dma_start(w8A[0][:], w8d[0])
        nc.sync.dma_start(x8_sb[:, 0], x8d[0])
        for j in range(1, NA):
            nc.sync.dma_start(w8A[j][:], w8d[j])
        nc.sync.dma_start(x8_sb[:, 1], x8d[1])
        nc.sync.dma_start(x8_sb[:, 2], x8d[2])
        nc.sync.dma_start(x8_sb[:, 3], x8d[3])
        for j in range(NA):
            nc.sync.dma_start(wmA[j][:], wmd[j])
        for f in range(4, F):
            nc.sync.dma_start(x8_sb[:, f], x8d[f])
        XBC = 3  # xb DMA chunk (k-tiles)
        for c0 in range(0, TB, XBC):
            c1 = min(c0 + XBC, TB)
            nc.sync.dma_start(xb_sb[:, c0:c1, :], xbd[:, c0:c1, :])

        def mm_dr(ps, wt, f, mc, start):
            nc.tensor.matmul(
                ps[:, :],
                wt[:, f, :, :],
                x8_sb[:, f, :, mc * 512:(mc + 1) * 512],
                start=start, stop=False, perf_mode=DR,
            )

        def mm_bf(ps, wt, t, mc, stop):
            nc.tensor.matmul(
                ps[:, :],
                wt[:, t, :],
                xb_sb[:, t, mc * 512:(mc + 1) * 512],
                start=False, stop=stop,
            )

        def drain2(psums, nt):
            # both m-chunks of one n-tile -> one SBUF tile -> one store on
            # the scalar (ACT) HWDGE ring; bf16(psum * 2*w_scale + bias)
            o = opool.tile([128, M], mybir.dt.bfloat16, tag="o", name=f"o{nt}")
            for mc in range(MT):
                nc.scalar.activation(
                    o[:, mc * 512:(mc + 1) * 512], psums[mc][:, :],
                    AF.Identity, bias=bias_sb[:, nt:nt + 1], scale=sc_sb[:, 0:1],
                )
            nc.scalar.dma_start(yT[nt * 128:(nt + 1) * 128, :], o[:])

        def drain1(ps, nt, mc):
            o = opool.tile([128, 512], mybir.dt.bfloat16, tag="oh", name=f"o{nt}_{mc}")
            nc.scalar.activation(
                o[:], ps[:, :],
                AF.Identity, bias=bias_sb[:, nt:nt + 1], scale=sc_sb[:, 0:1],
            )
            nc.scalar.dma_start(yT[nt * 128:(nt + 1) * 128, mc * 512:(mc + 1) * 512], o[:])

        # ---- Phase A: nt 0..NA-1, k-outer, paced by x arrival
        psA = {
            (j, mc): pspool.tile([128, 512], mybir.dt.float32, tag="ps", name=f"psA{j}_{mc}")
            for j in range(NA) for mc in range(MT)
        }
        for f in range(F):
            for j in range(NA):
                for mc in range(MT):
                    mm_dr(psA[j, mc], w8A[j], f, mc, start=(f == 0))
        for t in range(TB):
            for j in range(NA):
                for mc in range(MT):
                    mm_bf(psA[j, mc], wmA[j], t, mc, stop=(t == TB - 1))
        for j in range(NA):
            drain2([psA[j, 0], psA[j, 1]], j)

        # ---- Phase B: per n-tile, weights double-buffered
        for nt in range(NA, NT):
            w8t = wpool.tile([128, F, 2, 128], mybir.dt.float8e4, tag="w8b", name=f"w8_{nt}")
            wmt = wpool.tile([128, TB, 128], mybir.dt.float8e4, tag="wmb", name=f"wm_{nt}")
            nc.sync.dma_start(w8t[:], w8d[nt])
            nc.sync.dma_start(wmt[:], wmd[nt])
            last = nt == NT - 1
            psb = [
                pspool.tile([128, 512], mybir.dt.float32, tag="ps", name=f"ps{nt}_{mc}")
                for mc in range(MT)
            ]
            for mc in range(MT):
                for f in range(F):
                    mm_dr(psb[mc], w8t, f, mc, start=(f == 0))
                for t in range(TB):
                    mm_bf(psb[mc], wmt, t, mc, stop=(t == TB - 1))
                if last and mc == 0:
                    drain1(psb[mc], nt, mc)
                elif last:
                    # final drain split into 256-col chunks; the two stores
                    # ride different HWDGE rings so they overlap
                    oA = opool.tile([128, 256], mybir.dt.bfloat16, tag="ohA", name="ohA")
                    oB = opool.tile([128, 256], mybir.dt.bfloat16, tag="ohB", name="ohB")
                    nc.scalar.activation(
                        oA[:], psb[mc][:, 0:256],
                        AF.Identity, bias=bias_sb[:, nt:nt + 1], scale=sc_sb[:, 0:1],
                    )
                    nc.sync.dma_start(yT[nt * 128:(nt + 1) * 128, 512:768], oA[:])
                    nc.scalar.activation(
                        oB[:], psb[mc][:, 256:512],
                        AF.Identity, bias=bias_sb[:, nt:nt + 1], scale=sc_sb[:, 0:1],
                    )
                    nc.scalar.dma_start(yT[nt * 128:(nt + 1) * 128, 768:M], oB[:])
            if not last:
                drain2(psb, nt)
    nc.compile()
    return nc


def _prep_inputs(x, weight_f8, w_scale, bias):
    x2 = np.asarray(x)
    if x2.dtype != bf16:
        x2 = x2.astype(bf16)
    xm = x2.reshape(M, K)
    xT = np.ascontiguousarray(xm.T)  # [K, M] bf16

    # x fp8 part: [f, p, j, m] with k = f*256 + j*128 + p
    x8_dev = np.ascontiguousarray(
        xT[:KF].reshape(F, 2, 128, M).transpose(0, 2, 1, 3)
    ).astype(e4)  # [F, 128, 2, M]
    # x bf16 part: [p, t, m] with k = KF + t*128 + p
    xb_dev = np.ascontiguousarray(
        xT[KF:].reshape(TB, 128, M).transpose(1, 0, 2)
    )  # [128, TB, M]

    # weights: w_f8/2 is exactly fp8e4-representable (<=224); compensated by
    # scale 2*w_scale at drain. (Sub-subnormal tail rounds with max abs err
    # 2^-10 in w_f8 units - negligible.)
    wq = np.asarray(weight_f8, dtype=np.float32)
    w_half = (wq * np.float32(0.5)).astype(e4)  # [N, K] fp8

    s2 = np.float32(2.0) * np.float32(np.asarray(w_scale).astype(bf16))
    sc_dev = np.full((128, 1), s2, np.float32)

    bias_r = np.asarray(bias, dtype=np.float32).astype(bf16).astype(np.float32)

    in_maps = []
    for c in range(NC):
        part = w_half[c * NPER:(c + 1) * NPER]  # [1792, 4096] fp8
        w8_dev = np.ascontiguousarray(
            part[:, :KF].reshape(NT, 128, F, 2, 128).transpose(0, 4, 2, 3, 1)
        )  # [NT, p, F, j, n2]
        wm_dev = np.ascontiguousarray(
            part[:, KF:].reshape(NT, 128, TB, 128).transpose(0, 3, 2, 1)
        )  # [NT, p, TB, n2]
        bias_grid = np.ascontiguousarray(
            bias_r[c * NPER:(c + 1) * NPER].reshape(NT, 128).T
        )  # [128, NT]
        in_maps.append({
            "x8": x8_dev, "xb": xb_dev, "w8": w8_dev, "wm": wm_dev,
            "bias": bias_grid, "sc": sc_dev,
        })
    return in_maps


def run(x, weight_f8, w_scale, bias, trace=False, tmpdir=None):
    from concourse.bass_utils import run_bass_kernel_spmd

    if "nc" not in _cache:
        _cache["nc"] = _build_nc()
    nc = _cache["nc"]
    in_maps = _prep_inputs(x, weight_f8, w_scale, bias)
    res = run_bass_kernel_spmd(
        nc, in_maps, list(range(NC)), trace=trace, tmpdir=tmpdir
    )
    parts = [np.asarray(res.results[c]["yT"]) for c in range(NC)]  # each [1792, 1024]
    y = np.ascontiguousarray(np.concatenate(parts, axis=0).T)  # [1024, 14336]
    return y.reshape(2, 512, N), res


def kernel(x, weight_f8, w_scale, bias):
    y, _ = run(x, weight_f8, w_scale, bias)
    return y


# revision 18
# speedup vs baseline: 1.1904x; 1.0014x over previous
"""F8Linear as a column-parallel hybrid fp8/bf16 GEMM across 8 NeuronCores.

y = x @ (w_f8 * w_scale).T + bias
  x: [2, 512, 4096] bf16, w_f8: [14336, 4096] f32 (fp8-representable values),
  w_scale: scalar f32, bias: [14336] f32 -> y: [2, 512, 14336] bf16

Sharding: column-parallel - each core owns 1792 out-features; x replicated;
host gathers the 8 output slices. No collectives.

Speed trick: the TRN2 PE runs fp8e4 matmuls in DoubleRow mode at 2x the bf16
k-throughput (measured 220ns per FD=512 matmul contracting 256 k, vs 215ns
per 128 k in bf16). The weights are exactly fp8-representable: w_f8/2 lands
in TRN fp8e4 range (|w|<=224<240), so the weight side is EXACT in fp8 and
one fp32 scale (2*w_scale) is applied at PSUM drain. Only x must be
quantized to e4m3 for the DoubleRow operand, which costs ~2.7% rms error if
applied to all of K - too much for the 2e-2 gate. So K is split: the first
F=7 k-tiles of 256 (1792 k) run as fp8 DoubleRow; the remaining 18 k-tiles
of 128 (2304 k) run with x in bf16 against fp8e4 stationary weights (mixed
dtype runs at full bf16 rate and is exact). Measured vs the seeded
reference: max-rel 0.0170 / rms-rel 0.0181 at F=7.

All weights ship as fp8 (7.2 MB/core vs 14.7 bf16); x ships as 1.75 MB fp8 +
4.5 MB bf16. Per (n-tile, m-chunk) PSUM group: 7 DR + 18 bf16 matmuls
(~5.4us); 28 groups ~= 151us PE stream vs 191us for pure bf16.

Structure mirrors the tuned bf16 baseline: PE warmup during entry preamble,
phase A (first 4 n-tiles, k-outer, paced by x arrival), phase B per
(n-tile) with double-buffered weight streaming. Input DMAs ride the sync
HWDGE ring; output stores ride the scalar (ACT) HWDGE ring so they never
block inbound traffic. Drain = one ScalarE activation: bf16(psum*s2 + bias).
"""

import numpy as np
import ml_dtypes

bf16 = ml_dtypes.bfloat16
e4 = ml_dtypes.float8_e4m3  # TRN FP8_EXP4 semantics (max normal 240)

NC = 8
M, K, N = 1024, 4096, 14336
NPER = N // NC  # 1792 out-features per core
NT = NPER // 128  # 14 n-tiles
F = 8  # DoubleRow k-tiles of 256 (fp8 x); k in [0, 256*F)
KF = 256 * F
TB = (K - KF) // 128  # 18 bf16 k-tiles of 128
NA = 4  # phase-A n-tiles (4 nt x 2 m-chunks = 8 PSUM banks)
MT = 2  # m-chunks of 512

_cache = {}


def _build_nc():
    import concourse.bacc as bacc
    import concourse.mybir as mybir
    import concourse.tile as tile
    from contextlib import ExitStack

    DR = mybir.MatmulPerfMode.DoubleRow
    AF = mybir.ActivationFunctionType

    nc = bacc.Bacc("TRN2", target_bir_lowering=False, debug=False)
    x8d = nc.declare_dram_parameter("x8", [F, 128, 2, M], mybir.dt.float8e4, isOutput=False)
    xbd = nc.declare_dram_parameter("xb", [128, TB, M], mybir.dt.bfloat16, isOutput=False)
    w8d = nc.declare_dram_parameter("w8", [NT, 128, F, 2, 128], mybir.dt.float8e4, isOutput=False)
    wmd = nc.declare_dram_parameter("wm", [NT, 128, TB, 128], mybir.dt.float8e4, isOutput=False)
    # phase-A weights repacked partition-major so one DMA spans several n-tiles
    wa8d = nc.declare_dram_parameter("wa8", [128, NA, F, 2, 128], mybir.dt.float8e4, isOutput=False)
    wamd = nc.declare_dram_parameter("wam", [128, NA, TB, 128], mybir.dt.float8e4, isOutput=False)
    bgd = nc.declare_dram_parameter("bias", [128, NT], mybir.dt.float32, isOutput=False)
    scd = nc.declare_dram_parameter("sc", [128, 1], mybir.dt.float32, isOutput=False)
    yT = nc.declare_dram_parameter("yT", [NPER, M], mybir.dt.bfloat16, isOutput=True)

    with tile.TileContext(nc) as tc, ExitStack() as ctx:
        xpool = ctx.enter_context(tc.tile_pool(name="x", bufs=1))
        wapool = ctx.enter_context(tc.tile_pool(name="wa", bufs=1))
        wpool = ctx.enter_context(tc.tile_pool(name="w", bufs=3))
        bpool = ctx.enter_context(tc.tile_pool(name="b", bufs=1))
        opool = ctx.enter_context(tc.tile_pool(name="o", bufs=4))
        pspool = ctx.enter_context(tc.tile_pool(name="ps", bufs=8, space="PSUM"))

        # PE warmup: dep-free dummy matmuls run during the entry preamble +
        # first-DMA wait (~10us before real operands land) and burn the
        # cold-clock HAM window on garbage instead of real work.
        scratch = nc.alloc_sbuf_tensor("warm_src", [128, 128], mybir.dt.bfloat16)
        ps_warm = pspool.tile([128, 128], mybir.dt.float32, tag="ps")
        for _ in range(42):
            nc.tensor.matmul(ps_warm[:, :], scratch[:, :], scratch[:, :], start=True, stop=True)

        bias_sb = bpool.tile([128, NT], mybir.dt.float32)
        sc_sb = bpool.tile([128, 1], mybir.dt.float32, tag="sc")
        nc.gpsimd.dma_start(bias_sb[:], bgd[:])
        nc.gpsimd.dma_start(sc_sb[:], scd[:])

        x8_sb = xpool.tile([128, F, 2, M], mybir.dt.float8e4, tag="x8")
        xb_sb = xpool.tile([128, TB, M], mybir.dt.bfloat16, tag="xb")
        wa8 = wapool.tile([128, NA, F, 2, 128], mybir.dt.float8e4, tag="wa8")
        wam = wapool.tile([128, NA, TB, 128], mybir.dt.float8e4, tag="wam")

        # ---- input DMA issue order (sync ring), matched to PE consumption.
        # Each dma_start occupies the sequencer ~0.65us, so >=256KB chunks;
        # concurrent input DMA on the scalar ring slows the PE stream ~19%
        # (SBUF port contention) - keep all inputs on the sync ring.
        nc.sync.dma_start(wa8[:, 0:2], wa8d[:, 0:2])
        nc.sync.dma_start(x8_sb[:, 0], x8d[0])
        nc.sync.dma_start(wa8[:, 2:4], wa8d[:, 2:4])
        nc.sync.dma_start(x8_sb[:, 1], x8d[1])
        nc.sync.dma_start(x8_sb[:, 2], x8d[2])
        nc.sync.dma_start(wam[:, 0:2], wamd[:, 0:2])
        nc.sync.dma_start(x8_sb[:, 3], x8d[3])
        nc.sync.dma_start(wam[:, 2:4], wamd[:, 2:4])
        for f in range(4, F):
            nc.sync.dma_start(x8_sb[:, f], x8d[f])
        XBC = 3  # xb DMA chunk (k-tiles)
        for c0 in range(0, TB, XBC):
            c1 = min(c0 + XBC, TB)
            nc.sync.dma_start(xb_sb[:, c0:c1, :], xbd[:, c0:c1, :])

        def mm_dr(ps, w3, f, mc, start):
            nc.tensor.matmul(
                ps[:, :],
                w3,
                x8_sb[:, f, :, mc * 512:(mc + 1) * 512],
                start=start, stop=False, perf_mode=DR,
            )

        def mm_bf(ps, w2, t, mc, stop):
            nc.tensor.matmul(
                ps[:, :],
                w2,
                xb_sb[:, t, mc * 512:(mc + 1) * 512],
                start=False, stop=stop,
            )

        def drain2(psums, nt):
            # both m-chunks of one n-tile -> one SBUF tile -> one store on
            # the scalar (ACT) HWDGE ring; bf16(psum * 2*w_scale + bias)
            o = opool.tile([128, M], mybir.dt.bfloat16, tag="o", name=f"o{nt}")
            for mc in range(MT):
                nc.scalar.activation(
                    o[:, mc * 512:(mc + 1) * 512], psums[mc][:, :],
                    AF.Identity, bias=bias_sb[:, nt:nt + 1], scale=sc_sb[:, 0:1],
                )
            nc.scalar.dma_start(yT[nt * 128:(nt + 1) * 128, :], o[:])

        def drain1(ps, nt, mc):
            o = opool.tile([128, 512], mybir.dt.bfloat16, tag="oh", name=f"o{nt}_{mc}")
            nc.scalar.activation(
                o[:], ps[:, :],
                AF.Identity, bias=bias_sb[:, nt:nt + 1], scale=sc_sb[:, 0:1],
            )
            nc.scalar.dma_start(yT[nt * 128:(nt + 1) * 128, mc * 512:(mc + 1) * 512], o[:])

        # ---- Phase A: nt 0..NA-1, k-outer, paced by x arrival
        psA = {
            (j, mc): pspool.tile([128, 512], mybir.dt.float32, tag="ps", name=f"psA{j}_{mc}")
            for j in range(NA) for mc in range(MT)
        }
        for f in range(F):
            for j in range(NA):
                for mc in range(MT):
                    mm_dr(psA[j, mc], wa8[:, j, f, :, :], f, mc, start=(f == 0))
        for t in range(TB):
            for j in range(NA):
                for mc in range(MT):
                    mm_bf(psA[j, mc], wam[:, j, t, :], t, mc, stop=(t == TB - 1))
        for j in range(NA):
            drain2([psA[j, 0], psA[j, 1]], j)

        # ---- Phase B: per n-tile, weights double-buffered
        for nt in range(NA, NT):
            w8t = wpool.tile([128, F, 2, 128], mybir.dt.float8e4, tag="w8b", name=f"w8_{nt}")
            wmt = wpool.tile([128, TB, 128], mybir.dt.float8e4, tag="wmb", name=f"wm_{nt}")
            nc.sync.dma_start(w8t[:], w8d[nt])
            nc.sync.dma_start(wmt[:], wmd[nt])
            last = nt == NT - 1
            psb = [
                pspool.tile([128, 512], mybir.dt.float32, tag="ps", name=f"ps{nt}_{mc}")
                for mc in range(MT)
            ]
            for mc in range(MT):
                for f in range(F):
                    mm_dr(psb[mc], w8t[:, f, :, :], f, mc, start=(f == 0))
                for t in range(TB):
                    mm_bf(psb[mc], wmt[:, t, :], t, mc, stop=(t == TB - 1))
                if last and mc == 0:
                    drain1(psb[mc], nt, mc)
                elif last:
                    # final drain split into 256-col chunks; the two stores
                    # ride different HWDGE rings so they overlap
                    oA = opool.tile([128, 256], mybir.dt.bfloat16, tag="ohA", name="ohA")
                    oB = opool.tile([128, 256], mybir.dt.bfloat16, tag="ohB", name="ohB")
                    nc.scalar.activation(
                        oA[:], psb[mc][:, 0:256],
                        AF.Identity, bias=bias_sb[:, nt:nt + 1], scale=sc_sb[:, 0:1],
                    )
                    nc.sync.dma_start(yT[nt * 128:(nt + 1) * 128, 512:768], oA[:])
                    nc.scalar.activation(
                        oB[:], psb[mc][:, 256:512],
                        AF.Identity, bias=bias_sb[:, nt:nt + 1], scale=sc_sb[:, 0:1],
                    )
                    nc.scalar.dma_start(yT[nt * 128:(nt + 1) * 128, 768:M], oB[:])
            if not last:
                drain2(psb, nt)
    nc.compile()
    return nc


def _prep_inputs(x, weight_f8, w_scale, bias):
    x2 = np.asarray(x)
    if x2.dtype != bf16:
        x2 = x2.astype(bf16)
    xm = x2.reshape(M, K)
    xT = np.ascontiguousarray(xm.T)  # [K, M] bf16

    # x fp8 part: [f, p, j, m] with k = f*256 + j*128 + p
    x8_dev = np.ascontiguousarray(
        xT[:KF].reshape(F, 2, 128, M).transpose(0, 2, 1, 3)
    ).astype(e4)  # [F, 128, 2, M]
    # x bf16 part: [p, t, m] with k = KF + t*128 + p
    xb_dev = np.ascontiguousarray(
        xT[KF:].reshape(TB, 128, M).transpose(1, 0, 2)
    )  # [128, TB, M]

    # weights: w_f8/2 is exactly fp8e4-representable (<=224); compensated by
    # scale 2*w_scale at drain. (Sub-subnormal tail rounds with max abs err
    # 2^-10 in w_f8 units - negligible.)
    wq = np.asarray(weight_f8, dtype=np.float32)
    w_half = (wq * np.float32(0.5)).astype(e4)  # [N, K] fp8

    s2 = np.float32(2.0) * np.float32(np.asarray(w_scale).astype(bf16))
    sc_dev = np.full((128, 1), s2, np.float32)

    bias_r = np.asarray(bias, dtype=np.float32).astype(bf16).astype(np.float32)

    in_maps = []
    for c in range(NC):
        part = w_half[c * NPER:(c + 1) * NPER]  # [1792, 4096] fp8
        w8_dev = np.ascontiguousarray(
            part[:, :KF].reshape(NT, 128, F, 2, 128).transpose(0, 4, 2, 3, 1)
        )  # [NT, p, F, j, n2]
        wm_dev = np.ascontiguousarray(
            part[:, KF:].reshape(NT, 128, TB, 128).transpose(0, 3, 2, 1)
        )  # [NT, p, TB, n2]
        bias_grid = np.ascontiguousarray(
            bias_r[c * NPER:(c + 1) * NPER].reshape(NT, 128).T
        )  # [128, NT]
        wa8_dev = np.ascontiguousarray(w8_dev[:NA].transpose(1, 0, 2, 3, 4))
        wam_dev = np.ascontiguousarray(wm_dev[:NA].transpose(1, 0, 2, 3))
        in_maps.append({
            "x8": x8_dev, "xb": xb_dev, "w8": w8_dev, "wm": wm_dev,
            "wa8": wa8_dev, "wam": wam_dev,
            "bias": bias_grid, "sc": sc_dev,
        })
    return in_maps


def run(x, weight_f8, w_scale, bias, trace=False, tmpdir=None):
    from concourse.bass_utils import run_bass_kernel_spmd

    if "nc" not in _cache:
        _cache["nc"] = _build_nc()
    nc = _cache["nc"]
    in_maps = _prep_inputs(x, weight_f8, w_scale, bias)
    res = run_bass_kernel_spmd(
        nc, in_maps, list(range(NC)), trace=trace, tmpdir=tmpdir
    )
    parts = [np.asarray(res.results[c]["yT"]) for c in range(NC)]  # each [1792, 1024]
    y = np.ascontiguousarray(np.concatenate(parts, axis=0).T)  # [1024, 14336]
    return y.reshape(2, 512, N), res


def kernel(x, weight_f8, w_scale, bias):
    y, _ = run(x, weight_f8, w_scale, bias)
    return y
